# revision 17
# baseline (speedup 1.0000x reference)
"""v11: v6 + junk warm burst 9->6 matmuls: the six e7-free Z matmuls run before
y0's last k-pair, filling the wait for the final exp."""

import numpy as np
import ml_dtypes

import concourse.bass as bass
import concourse.tile as tile
from concourse import bacc, mybir
from concourse import bass_utils

F32 = mybir.dt.float32
BF16 = mybir.dt.bfloat16
F8 = mybir.dt.float8e4
AF = mybir.ActivationFunctionType
DR = mybir.MatmulPerfMode.DoubleRow

N, C, H, W = 32, 512, 32, 32
D = C // 2
HW = H * W
NCORES = 8
NS = N // NCORES  # samples per core
P = 128
CT = C // P   # 4 c-tiles
KT = HW // P  # 8 hw-tiles
MT_D = (2 * D) // P  # 4 m-tiles of combined theta/phi conv
NQ = HW // 512  # 2 free-dim halves
KP = KT // 2  # 4 k-pairs for DoubleRow over HW

TP_BOOST = 8.0     # on W_theta and W_phi (and their biases)
FU_BOOST = 16.0    # on W_fuse
EXP_SCALE = 1.0 / (TP_BOOST * TP_BOOST * np.sqrt(np.float32(D)))
EXP_BIAS = -3.0    # softmax-shift: keeps exp() within e4m3 range


def _emit(tc):
    nc = tc.nc

    x_f8 = nc.dram_tensor("x_f8", [NS, C, HW], F8, kind="ExternalInput").ap()
    x_res = nc.dram_tensor("x_res", [NS, C, HW], BF16, kind="ExternalInput").ap()
    wcat_t = nc.dram_tensor("wcat_t", [C, 2 * D], F8, kind="ExternalInput").ap()
    b_cat = nc.dram_tensor("b_cat", [2 * D, 1], F32, kind="ExternalInput").ap()
    wfu_t = nc.dram_tensor("wfu_t", [C, C], F8, kind="ExternalInput").ap()
    out_d = nc.dram_tensor("out", [NS, C, HW], BF16, kind="ExternalOutput").ap()

    import contextlib
    ctx = contextlib.ExitStack()
    with ctx:
        # ---- constant pools ----
        wpool = ctx.enter_context(tc.tile_pool(name="wpool", bufs=1))
        wcat_sb = wpool.tile([P, CT, 2 * D], F8)

        # ---- working pools ----
        xf8_pool = ctx.enter_context(tc.tile_pool(name="xf8", bufs=3))
        xres_pool = ctx.enter_context(tc.tile_pool(name="xres", bufs=2))
        tp_pool = ctx.enter_context(tc.tile_pool(name="tp", bufs=2))
        gt_pool = ctx.enter_context(tc.tile_pool(name="gt", bufs=2))
        e_pool = ctx.enter_context(tc.tile_pool(name="e", bufs=2))
        rz_pool = ctx.enter_context(tc.tile_pool(name="rz", bufs=2))
        fin_pool = ctx.enter_context(tc.tile_pool(name="fin", bufs=3))
        out_pool = ctx.enter_context(tc.tile_pool(name="outp", bufs=3))

        # PSUM: 1-buf resident pool for the kp-loop-accumulating y0 (+y3),
        # 3-buf rotating pool for everything else. 2 + 3x2 banks = all 8.
        psum_y = ctx.enter_context(tc.tile_pool(name="psy", bufs=1, space="PSUM"))
        psum_rot = ctx.enter_context(tc.tile_pool(name="psr", bufs=3, space="PSUM"))

        # HAM pre-warm + ACT exp-table pre-load during the initial DMA wait
        ones_f8 = wpool.tile([P, 2, P], F8)
        nc.vector.memset(ones_f8[:], FU_BOOST)  # folds the wfu boost out of Z
        warm_rhs = wpool.tile([P, 512], BF16)
        nc.vector.memset(warm_rhs[:], 0.0)
        ebias_sb = wpool.tile([P, 1], F32)
        nc.vector.memset(ebias_sb[:], EXP_BIAS)
        warm_e = wpool.tile([P, 16], F8)
        nc.scalar.activation(warm_e[:], warm_rhs[:, 0:16], AF.Exp,
                             bias=ebias_sb[:], scale=float(EXP_SCALE))
        ps_warm = psum_rot.tile([P, 512], F32, tag="mm", name="ps_warm")
        warm_lhs = wpool.tile([P, P], BF16)
        nc.vector.memset(warm_lhs[:], 0.0)
        for w in range(6):
            # accumulate: with start=True each, walrus dead-stores them.
            # 6 junk matmuls (not 9): the first conv k-pair's DMAs land at
            # ~10.6us, so the burst only needs to cover until then -- the
            # real matmuls continue the HAM busy window seamlessly
            nc.tensor.matmul(ps_warm[:], warm_lhs[:], warm_rhs[:],
                             start=(w == 0), stop=(w == 5))
        # read ps_warm so walrus can't dead-code-eliminate the HAM warmup
        warm_sink = wpool.tile([P, 8], F32)
        nc.vector.tensor_copy(warm_sink[:], ps_warm[:, 0:8])

        xf8_tiles = {}
        xf8_tiles[0] = xf8_pool.tile([P, CT, HW], F8, tag="xf8", name="xf80")
        for k in range(CT):
            nc.sync.dma_start(
                wcat_sb[:, k, :],
                wcat_t.rearrange("(t p) d -> t p d", p=P)[k],
            )
            nc.gpsimd.dma_start(
                xf8_tiles[0][:, k, :],
                x_f8[0].rearrange("(t p) f -> t p f", p=P)[k],
            )

        # bcat (tiny, needed by the first conv copies) first, then wfu on the
        # sync ring right behind wcat so sample 0's gt matmuls don't wait
        bcat_sb = wpool.tile([P, MT_D], F32)
        nc.sync.dma_start(
            bcat_sb.rearrange("p (t o) -> p t o", o=1),
            b_cat.rearrange("(t p) o -> p t o", p=P),
        )
        wfu_sb = wpool.tile([P, CT, C], F8)
        nc.sync.dma_start(
            wfu_sb[:],
            wfu_t.rearrange("(t p) d -> p t d", p=P),
        )
        def load_x(s):
            # one-sample-ahead prefetch of the fp8 x (sample 0 is chunked
            # above so the first conv matmuls start as chunks land)
            if s < NS and s not in xf8_tiles:
                xf8_tiles[s] = xf8_pool.tile(
                    [P, CT, HW], F8, tag="xf8", name=f"xf8{s}"
                )
                nc.sync.dma_start(
                    xf8_tiles[s][:],
                    x_f8[s].rearrange("(t p) f -> p t f", p=P),
                )

        tp_tiles = {}

        def conv(s):
            # ---- combined theta/phi 1x1 conv: tp = wcat.T @ x + b ----
            xf8_sb = xf8_tiles[s]
            tp_sb = tp_pool.tile([P, MT_D, HW], F8, tag="tp", name=f"tp{s}")
            tp_tiles[s] = tp_sb

            def conv_mm(ps, m, kp):
                for nq in range(NQ):
                    nc.tensor.matmul(
                        ps[:, nq * 512:(nq + 1) * 512],
                        wcat_sb[:, 2 * kp:2 * kp + 2, m * P:(m + 1) * P],
                        xf8_sb[:, 2 * kp:2 * kp + 2, nq * 512:nq * 512 + 512],
                        start=(kp == 0),
                        stop=(kp == CT // 2 - 1),
                        perf_mode=DR,
                    )

            def conv_copy(ps, m):
                # sample 0: DVE is idle at the kernel head -- splitting the
                # copies pulls s0's exp chain ~1.5us earlier
                if s == 0 and m % 2 == 0:
                    nc.vector.tensor_scalar_add(
                        tp_sb[:, m, :], ps[:], bcat_sb[:, m:m + 1])
                else:
                    nc.scalar.add(tp_sb[:, m, :], ps[:], bcat_sb[:, m:m + 1])

            if s == 0:
                # k-outer so each arriving x k-pair feeds all m immediately;
                # 4 live psums span both pools
                ps_cvs = [
                    (psum_y if m < 1 else psum_rot).tile(
                        [P, HW], F32, tag="mm", name=f"ps_cv0_{m}")
                    for m in range(MT_D)
                ]
                for kp in range(CT // 2):
                    for m in range(MT_D):
                        conv_mm(ps_cvs[m], m, kp)
                for m in range(MT_D):
                    conv_copy(ps_cvs[m], m)
            else:
                for m in range(MT_D):
                    ps_cv = psum_rot.tile(
                        [P, HW], F32, tag="mm", name=f"ps_cv{s}_{m}"
                    )
                    for kp in range(CT // 2):
                        conv_mm(ps_cv, m, kp)
                    conv_copy(ps_cv, m)

        conv(0)
        for s in range(NS):
            load_x(s)
            if s not in tp_tiles:
                conv(s)
            xf8_sb = xf8_tiles[s]
            tp_sb = tp_tiles[s]
            xres_sb = xres_pool.tile([P, CT, HW], BF16, tag="xres")
            nc.sync.dma_start(
                xres_sb[:],
                x_res[s].rearrange("(t p) f -> p t f", p=P),
            )

            gt_sb = gt_pool.tile([P, KT, C], F8, tag="gt")
            e_sb = e_pool.tile([P, KT, HW], F8, tag="e")

            def s_mtile(m):
                ps_s = psum_rot.tile([P, HW], F32, tag="mm", name=f"ps_s{s}_{m}")
                for nq in range(NQ):
                    nc.tensor.matmul(
                        ps_s[:, nq * 512:(nq + 1) * 512],
                        tp_sb[:, 2:4, m * P:(m + 1) * P],
                        tp_sb[:, 0:2, nq * 512:nq * 512 + 512],
                        start=True,
                        stop=True,
                        perf_mode=DR,
                    )
                nc.scalar.activation(
                    e_sb[:, m, :], ps_s[:], AF.Exp,
                    bias=ebias_sb[:], scale=float(EXP_SCALE),
                )

            def gt_pair(j):
                ps_g = psum_rot.tile([P, HW], F32, tag="mm", name=f"ps_g{s}_{j}")
                for mi in range(2):
                    m = 2 * j + mi
                    for kp in range(CT // 2):
                        nc.tensor.matmul(
                            ps_g[:, mi * C:(mi + 1) * C],
                            xf8_sb[:, 2 * kp:2 * kp + 2, m * P:(m + 1) * P],
                            wfu_sb[:, 2 * kp:2 * kp + 2, :],
                            start=(kp == 0),
                            stop=(kp == CT // 2 - 1),
                            perf_mode=DR,
                        )
                nc.vector.tensor_copy(gt_sb[:, 2 * j:2 * j + 2, :], ps_g[:])

            def y_kp(ps_y, m, kp):
                for nq in range(NQ):
                    nc.tensor.matmul(
                        ps_y[:, nq * 512:(nq + 1) * 512],
                        gt_sb[:, 2 * kp:2 * kp + 2, m * P:(m + 1) * P],
                        e_sb[:, 2 * kp:2 * kp + 2, nq * 512:nq * 512 + 512],
                        start=(kp == 0),
                        stop=(kp == KP - 1),
                        perf_mode=DR,
                        skip_group_check=True,
                    )

            def z_kp(ps_z, kp):
                for nq in range(NQ):
                    nc.tensor.matmul(
                        ps_z[:, nq * 512:(nq + 1) * 512],
                        ones_f8[:],
                        e_sb[:, 2 * kp:2 * kp + 2, nq * 512:nq * 512 + 512],
                        start=(kp == 0),
                        stop=(kp == KP - 1),
                        perf_mode=DR,
                        skip_group_check=True,
                    )

            ps_y0 = psum_y.tile([P, HW], F32, tag="mm", name=f"ps_y{s}_0")
            for kp in range(KP):
                gt_pair(kp)
                s_mtile(2 * kp)
                s_mtile(2 * kp + 1)
                if kp > 0:
                    y_kp(ps_y0, 0, kp - 1)

            # Z k-pairs 0-2 need only e0..e5 -- run them while waiting for
            # the final exp (e7), which gates y0's last k-pair and Z's own
            ps_z = psum_rot.tile([P, HW], F32, tag="mm", name=f"ps_z{s}")
            for kp in range(KP - 1):
                z_kp(ps_z, kp)
            y_kp(ps_y0, 0, KP - 1)
            z_kp(ps_z, KP - 1)

            def y_mtile(m, pool):
                ps_y = pool.tile([P, HW], F32, tag="mm", name=f"ps_y{s}_{m}")
                for kp in range(KP):
                    y_kp(ps_y, m, kp)
                return ps_y

            ps_y1 = y_mtile(1, psum_rot)

            rzb_sb = rz_pool.tile([P, HW], F32, tag="rz")
            nc.vector.reciprocal_approx_fast(out=rzb_sb[:], in_=ps_z[:])

            t1s = {}

            def cmul(m, ps_y):
                t1 = fin_pool.tile([P, HW], BF16, tag="fin", name=f"t1_{s}_{m}")
                t1s[m] = t1
                nc.vector.tensor_mul(t1[:], ps_y[:], rzb_sb[:])

            def cadd(m):
                o_sb = out_pool.tile([P, HW], BF16, tag="o", name=f"o_{s}_{m}")
                nc.vector.tensor_add(o_sb[:], t1s[m][:], xres_sb[:, m, :])
                nc.sync.dma_start(
                    out_d[s].rearrange("(t p) f -> t p f", p=P)[m],
                    o_sb[:],
                )

            cmul(0, ps_y0)
            cadd(0)
            ps_y2 = y_mtile(2, psum_rot)
            cmul(1, ps_y1)
            cadd(1)
            ps_y3 = y_mtile(3, psum_y)
            cmul(2, ps_y2)
            cadd(2)
            if s == NS - 1:
                t1 = fin_pool.tile([P, HW], BF16, tag="fin", name=f"t1_{s}_3")
                o_sb = out_pool.tile([P, HW], BF16, tag="o", name=f"o_{s}_3")
                for h in range(2):
                    hs = slice(h * 512, (h + 1) * 512)
                    nc.vector.tensor_mul(t1[:, hs], ps_y3[:, hs], rzb_sb[:, hs])
                    nc.vector.tensor_add(
                        o_sb[:, hs], t1[:, hs], xres_sb[:, CT - 1, hs])
                    nc.sync.dma_start(
                        out_d[s].rearrange("(t p) f -> t p f", p=P)[CT - 1][:, hs],
                        o_sb[:, hs],
                    )
            else:
                cmul(3, ps_y3)
                cadd(3)


_CACHE = {}


def _build():
    if "nc" not in _CACHE:
        nc = bacc.Bacc("TRN2", target_bir_lowering=False, debug=False)
        with tile.TileContext(nc) as tc:
            _emit(tc)
        nc.compile()
        _CACHE["nc"] = nc
    return _CACHE["nc"]


def _prep_in_maps(x, W_theta, b_theta, W_phi, b_phi, W_fuse, b_fuse):
    bf = ml_dtypes.bfloat16
    f8 = ml_dtypes.float8_e4m3
    xf = np.ascontiguousarray(x.reshape(N, C, HW).astype(np.float32))
    x_f8 = xf.astype(f8)
    x_res = (xf + b_fuse.astype(np.float32)[None, :, None]).astype(bf)
    wcat_t = np.ascontiguousarray(
        np.concatenate([W_theta.astype(np.float32) * TP_BOOST,
                        W_phi.astype(np.float32) * TP_BOOST], axis=0).T
    ).astype(f8)
    b_cat = np.concatenate([b_theta.astype(np.float32) * TP_BOOST,
                            b_phi.astype(np.float32) * TP_BOOST]).reshape(2 * D, 1)
    wfu_t = np.ascontiguousarray(
        W_fuse.astype(np.float32).T * FU_BOOST
    ).astype(f8)

    in_maps = []
    for c in range(NCORES):
        sl = slice(c * NS, (c + 1) * NS)
        in_maps.append({
            "x_f8": np.ascontiguousarray(x_f8[sl]),
            "x_res": np.ascontiguousarray(x_res[sl]),
            "wcat_t": wcat_t,
            "b_cat": b_cat.astype(np.float32),
            "wfu_t": wfu_t,
        })
    return in_maps


def _run(inputs, trace=False, **kw):
    nc = _build()
    in_maps = _prep_in_maps(**inputs)
    res = bass_utils.run_bass_kernel_spmd(
        nc, in_maps, core_ids=list(range(NCORES)), trace=trace, **kw
    )
    out = np.concatenate(
        [res.results[c]["out"].astype(np.float32) for c in range(NCORES)], axis=0
    )
    return out.reshape(N, C, H, W), res


def kernel(**inputs):
    inputs = {k: np.asarray(v) for k, v in inputs.items()}
    out, _ = _run(inputs, trace=False)
    return out
